# revision 72
# baseline (speedup 1.0000x reference)
"""3-layer GAT on 8 Trainium2 NeuronCores (v4).

Strategy (dst-sharded, edge-parallel within core):
- Host: add self-loops, sort edges by dst, partition dst nodes into 8
  contiguous shards (3840/core, 30 blocks of 128). Per dst-block, pad the
  edge list to a uniform chunk count (128 edges/chunk).
- Host precomputes the per-chunk one-hot masks onehot[e,d] (f8, exact 0/1)
  and their transposes onehotT[d,e]; both stream from DRAM (static DMA).
  No per-chunk is_equal builds on device.
- Device, per layer: phase A computes h_ext = x @ [W | wsrc | wdst] for the
  core's own node shard. The message-table row (1792B for layers 1/2) holds
  h head-blocks in f16 with interleaved 1.0 columns plus asrc; adst for the
  core's own dst nodes stays in SBUF (att_all tile).
- The message table is AllGathered into a pair-Shared DRAM tensor (~2x the
  bandwidth of Local-output collectives, one copy per HBM pair) in
  per-layer slices (SLICES_L); slices for layer l+1 are issued from inside
  scatter l right after the contributing phase-A blocks complete, so the
  collectives hide under the scatter. tabfull uses a slice-blocked row
  layout; the host remaps src indices per layer accordingly.
- Per-edge adst (pse_all) = onehotT.T @ att_blk, ALL chunks precomputed in
  the AllGather shadow between scatters (3-col matmuls, psum -> f16 SBUF).
- Scatter: per 8-chunk group, dma_gather the src rows (4 SWDGE queues,
  round-robin); ex = exp(leakyrelu(asrc+pse)) per group; per 128-edge chunk
  ONE stride-0-broadcast DVE multiply builds m3[e,(h,d)] = onehot*ex_h for
  all 3 heads, then psum_h += m3_h.T @ rows[h-block|1]. den comes from the
  interleaved ones column. Normalize, +bias, ELU; transpose via PE into
  f16 hT for the next layer's phase A.
"""
import sys, os
sys.path.insert(0, "/opt/trn_rl_repo")
import math
import numpy as np

from concourse import bacc, tile, mybir
from concourse.bass_utils import run_bass_kernel_spmd

F32 = mybir.dt.float32
F16 = mybir.dt.float16
F8 = mybir.dt.float8e4
I16 = mybir.dt.int16
I32 = mybir.dt.int32
AF = mybir.ActivationFunctionType
ALU = mybir.AluOpType

NEG_SLOPE = 0.2
# phase-A block ranges per AllGather slice, per layer
SLICES_L = {
    1: [0, 6, 12, 18, 24, 30],
    2: [0, 2, 5, 9, 13, 17, 21, 25, 28, 30],
    3: [0, 20, 28, 30],
}


def make_cfg(N, F_IN, H, C, OC, NCORES=8):
    cfg = dict(N=N, F_IN=F_IN, H=H, C=C, OC=OC, NCORES=NCORES)
    D1 = H * C
    blk = 128
    npc = math.ceil(N / (NCORES * blk)) * blk  # nodes per core
    cfg.update(
        D1=D1,
        BLK=blk,
        NPC=npc,
        NBLK=npc // blk,
        NPAD=npc * NCORES,
        TABW=896,        # layers 1/2 row: f16 elems (1792B)
        TAB3W=128,       # layer 3 row: f16 elems (256B)
        ASRC_OFF=771,    # f16 offset of asrc in the l1/l2 row
        A3_OFF=99,       # f16 offset of asrc in the l3 row
        PA_W=D1 + 6,     # phase-A psum width (h | asrc | adst)
        PA3_W=H * OC + 6,
    )
    return cfg


CFG_FULL = make_cfg(N=30000, F_IN=128, H=3, C=256, OC=32)


# ---------------------------------------------------------------- host prep
def prep_host(x, edge_index, Ws, asrcs, adsts, bs, cfg):
    """Returns (in_maps, nchunk)."""
    N, H, C, OC = cfg["N"], cfg["H"], cfg["C"], cfg["OC"]
    NCORES, NPC, NBLK, BLK = cfg["NCORES"], cfg["NPC"], cfg["NBLK"], cfg["BLK"]

    src = edge_index[0].astype(np.int64)
    dst = edge_index[1].astype(np.int64)
    loop = np.arange(N, dtype=np.int64)
    src = np.concatenate([src, loop])
    dst = np.concatenate([dst, loop])

    order = np.argsort(dst, kind="stable")
    src_s, dst_s = src[order], dst[order]
    gb = dst_s // BLK                       # global block id
    nblk_g = NCORES * NBLK
    counts = np.bincount(gb, minlength=nblk_g)
    nchunk = max(1, int(math.ceil(counts.max() / BLK)))
    spb = nchunk * BLK                      # slots per block
    slots = NBLK * spb                      # per core
    ncht = NBLK * nchunk

    offsets = np.zeros(nblk_g, np.int64)
    offsets[1:] = np.cumsum(counts)[:-1]
    pos_in_block = np.arange(len(dst_s)) - offsets[gb]
    core_id = gb // NBLK
    loc_blk = gb % NBLK

    # slice-blocked tabfull row remap: node g = c*NPC + b*128 + r ->
    # row = base[s] + c*(len_s*128) + (b-B_s)*BLK + r   (per-layer slices)
    def make_remap(SLICES):
        nsl = len(SLICES) - 1
        slice_of = np.zeros(NBLK, np.int64)
        base = np.zeros(nsl, np.int64)
        slen = np.zeros(nsl, np.int64)
        for s in range(nsl):
            slice_of[SLICES[s]:SLICES[s + 1]] = s
            base[s] = SLICES[s] * NCORES * BLK
            slen[s] = SLICES[s + 1] - SLICES[s]
        sl_arr = np.asarray(SLICES, np.int64)

        def remap(g):
            c = g // NPC
            b = (g % NPC) // BLK
            r = g % BLK
            s = slice_of[b]
            return base[s] + c * (slen[s] * BLK) + (b - sl_arr[s]) * BLK + r
        return remap

    remaps = {l: make_remap(SLICES_L[l]) for l in (1, 2, 3)}

    slot = loc_blk * spb + pos_in_block

    def wrap16(a):
        return np.ascontiguousarray(np.tile(a.reshape(-1, 16).T, (8, 1)))

    in_maps = []
    for k in range(NCORES):
        sel = core_id == k
        sl = slot[sel]
        relf = np.full(slots, 999, np.int64)  # padded: no match
        relf[sl] = dst_s[sel] % BLK
        srcs = []
        for l in (1, 2, 3):
            srcf = np.zeros(slots, np.int64)
            srcf[sl] = remaps[l](src_s[sel])
            srcs.append(wrap16(srcf.astype(np.int16)))
        srcidx3 = np.ascontiguousarray(np.stack(srcs, axis=1))  # [128,3,slots//16]

        # one-hot masks, f8e4m3 exact 0/1 (1.0 = 0x38): onehot[e,d], onehotT[d,e]
        import ml_dtypes
        oh = (relf.reshape(ncht, BLK)[:, :, None] ==
              np.arange(BLK)[None, None, :])  # [chunk, e, d]
        oh_e = np.ascontiguousarray(
            np.where(oh.transpose(1, 0, 2), np.uint8(0x38), np.uint8(0))
        ).view(ml_dtypes.float8_e4m3)
        oh_d = np.ascontiguousarray(
            np.where(oh.transpose(2, 0, 1), np.uint8(0x38), np.uint8(0))
        ).view(ml_dtypes.float8_e4m3)

        xk = np.zeros((NPC, cfg["F_IN"]), np.float32)
        lo, hi = k * NPC, min((k + 1) * NPC, N)
        if hi > lo:
            xk[: hi - lo] = x[lo:hi]
        xT = np.ascontiguousarray(
            xk.reshape(NBLK, BLK, cfg["F_IN"]).transpose(2, 0, 1)
        ).astype(np.float16)

        m = dict(
            xT_in=xT,
            srcidx_in=srcidx3,
            oh_e_in=oh_e,
            oh_d_in=oh_d,
        )
        in_maps.append(m)

    # weights (replicated): wx = [W | wsrc | wdst] in f16
    def wx(W, a_s, a_d):
        Ch = W.shape[1] // H
        Wr = W.reshape(W.shape[0], H, Ch)
        ws = np.einsum("khc,hc->kh", Wr, a_s)
        wd = np.einsum("khc,hc->kh", Wr, a_d)
        return np.concatenate([W, ws, wd], axis=1).astype(np.float16)

    wx1 = wx(Ws[0], asrcs[0], adsts[0])
    wx2 = wx(Ws[1], asrcs[1], adsts[1])
    wx3 = wx(Ws[2], asrcs[2], adsts[2])
    b1 = np.broadcast_to(bs[0], (128, cfg["D1"])).astype(np.float16).copy()
    b2 = np.broadcast_to(bs[1], (128, cfg["D1"])).astype(np.float16).copy()
    b3 = np.broadcast_to(bs[2], (128, OC)).astype(np.float32).copy()
    for m in in_maps:
        m.update(wx1_in=wx1, wx2_in=wx2, wx3_in=wx3, b1_in=b1, b2_in=b2, b3_in=b3)
    return in_maps, nchunk


# ------------------------------------------------------------- device build
def build_program(cfg, nchunk, ggrp=16, nq=4):
    N, F_IN, H, C, OC = cfg["N"], cfg["F_IN"], cfg["H"], cfg["C"], cfg["OC"]
    D1, NCORES, NPC, NBLK, BLK = (
        cfg["D1"], cfg["NCORES"], cfg["NPC"], cfg["NBLK"], cfg["BLK"])
    NPAD = cfg["NPAD"]
    TABW, TAB3W = cfg["TABW"], cfg["TAB3W"]
    ASRC_OFF, A3_OFF = cfg["ASRC_OFF"], cfg["A3_OFF"]
    PA_W, PA3_W = cfg["PA_W"], cfg["PA3_W"]
    CH1 = C + 1                     # f16 head stride in l1/l2 row
    CH3 = OC + 1                    # f16 head stride in l3 row
    ncht = NBLK * nchunk
    slots = ncht * BLK
    n_cin = D1 // 128
    ngrp = math.ceil(ncht / ggrp)   # gather groups per layer

    nc = bacc.Bacc("TRN2", target_bir_lowering=False, debug=False,
                   num_devices=NCORES, num_swdge_queues=nq)
    qrr = [0]

    def next_q():
        q = qrr[0] % nq
        qrr[0] += 1
        return q

    # ---- I/O
    xT_in = nc.dram_tensor("xT_in", [128, NBLK, F_IN], F16, kind="ExternalInput")
    srcidx_in = nc.dram_tensor("srcidx_in", [128, 3, slots // 16], I16, kind="ExternalInput")
    oh_e_in = nc.dram_tensor("oh_e_in", [128, ncht, BLK], F8, kind="ExternalInput")
    oh_d_in = nc.dram_tensor("oh_d_in", [128, ncht, BLK], F8, kind="ExternalInput")
    wx1_in = nc.dram_tensor("wx1_in", [F_IN, PA_W], F16, kind="ExternalInput")
    wx2_in = nc.dram_tensor("wx2_in", [D1, PA_W], F16, kind="ExternalInput")
    wx3_in = nc.dram_tensor("wx3_in", [D1, PA3_W], F16, kind="ExternalInput")
    b1_in = nc.dram_tensor("b1_in", [128, D1], F16, kind="ExternalInput")
    b2_in = nc.dram_tensor("b2_in", [128, D1], F16, kind="ExternalInput")
    b3_in = nc.dram_tensor("b3_in", [128, OC], F32, kind="ExternalInput")
    out_ext = nc.dram_tensor("out", [NPC, OC], F32, kind="ExternalOutput")

    # ---- DRAM scratch: per-slice shard inputs, pair-shared gathered tables
    def mk_tabsh(l, w):
        SL = SLICES_L[l]
        return [nc.dram_tensor(f"tabsh{l}_{s}",
                               [(SL[s + 1] - SL[s]) * BLK, w], F16)
                for s in range(len(SL) - 1)]

    tabsh = [mk_tabsh(1, TABW), mk_tabsh(2, TABW), mk_tabsh(3, TAB3W)]
    tabfull = [nc.dram_tensor("tabfull1", [NPAD, TABW], F16, addr_space="Shared"),
               nc.dram_tensor("tabfull2", [NPAD, TABW], F16, addr_space="Shared"),
               nc.dram_tensor("tabfull3", [NPAD, TAB3W], F16, addr_space="Shared")]
    RG = [list(range(NCORES))]

    with tile.TileContext(nc) as tc:
        with (
            tc.tile_pool(name="const", bufs=1) as cpool,
            tc.tile_pool(name="rows", bufs=2) as rpool,
            tc.tile_pool(name="oh", bufs=2) as opool,
            tc.tile_pool(name="work", bufs=2) as wpool,
            tc.tile_pool(name="psA", bufs=1, space="PSUM") as psA,
            tc.tile_pool(name="psH", bufs=1, space="PSUM") as psH,
            tc.tile_pool(name="psT", bufs=1, space="PSUM") as psT,
            tc.tile_pool(name="psE", bufs=2, space="PSUM") as psE,
            tc.tile_pool(name="m3p", bufs=2) as m3p,
            tc.tile_pool(name="ohd", bufs=1) as ohdp,
        ):
            # ---------------- constants
            def load_const(name, dram, shape, dtype):
                t = cpool.tile(shape, dtype, tag=name)
                nc.sync.dma_start(out=t[...], in_=dram[...])
                return t

            xT = load_const("xT", xT_in, [128, NBLK, F_IN], F16)
            srcidx = load_const("srcidx", srcidx_in, [128, 3, slots // 16], I16)
            b_sb = [load_const("b1", b1_in, [128, D1], F16),
                    load_const("b2", b2_in, [128, D1], F16),
                    load_const("b3", b3_in, [128, OC], F32)]
            wx1 = cpool.tile([128, 1, PA_W], F16, tag="wx1")
            nc.sync.dma_start(out=wx1[:, 0, :], in_=wx1_in[0:128, :])
            wx2 = cpool.tile([128, n_cin, PA_W], F16, tag="wx2")
            wx3 = cpool.tile([128, n_cin, PA3_W], F16, tag="wx3")
            for ct in range(n_cin):
                nc.sync.dma_start(out=wx2[:, ct, :], in_=wx2_in[ct * 128:(ct + 1) * 128, :])
                nc.sync.dma_start(out=wx3[:, ct, :], in_=wx3_in[ct * 128:(ct + 1) * 128, :])

            iota_i = cpool.tile([128, 128], I32, tag="iota_i")
            nc.gpsimd.iota(iota_i[...], pattern=[[1, 128]], base=0, channel_multiplier=0)
            iota16 = cpool.tile([128, 128], F16, tag="iota16")
            nc.vector.tensor_copy(iota16[...], iota_i[...])
            pidx_i = cpool.tile([128, 1], I32, tag="pidx_i")
            nc.gpsimd.iota(pidx_i[...], pattern=[[0, 1]], base=0, channel_multiplier=1)
            pidx_f = cpool.tile([128, 1], F32, tag="pidx_f")
            nc.vector.tensor_copy(pidx_f[...], pidx_i[...])
            ident16 = cpool.tile([128, 128], F16, tag="ident16")
            nc.vector.tensor_scalar(out=ident16[...], in0=iota16[...],
                                    scalar1=pidx_f[:, 0:1], scalar2=None,
                                    op0=ALU.is_equal)

            # persistent f16 transposed activations for next layer's phase A
            hT = cpool.tile([128, n_cin * NBLK, 128], F16, tag="hT")
            # per-layer adst of the core's own dst blocks [dstrow, (block, h)]
            att_all = [cpool.tile([128, NBLK, 3], F16, tag=f"att{l}",
                                  name=f"att{l}") for l in (1, 2, 3)]
            # per-edge adst for the CURRENT layer, precomputed between scatters
            pse_all = cpool.tile([128, ncht, 3], F16, tag="pse_all")

            # pse_all[e, c, h] = sum_d onehotT[d, c, e] * att[d, c//nchunk, h]
            def adst_precompute(l):
                att_l = att_all[l - 1]
                CPR = 128  # chunks per psum round (128*3 = 384 f32 <= bank)
                for r0 in range(0, ncht, CPR):
                    rn = min(CPR, ncht - r0)
                    ohd = ohdp.tile([128, CPR, BLK], F8, tag="ohd_r")
                    nc.scalar.dma_start(out=ohd[:, 0:rn, :],
                                        in_=oh_d_in[:, r0:r0 + rn, :])
                    pp = psE.tile([128, CPR, 3], F32, tag="pse_ps")
                    for i in range(rn):
                        j = (r0 + i) // nchunk
                        nc.tensor.matmul(pp[:, i, :], ohd[:, i, :],
                                         att_l[:, j, :], start=True, stop=True)
                    nc.scalar.copy(pse_all[:, r0:r0 + rn, :], pp[:, 0:rn, :])

            # ---------------- phase A
            def phaseA_tail(l, t, psum):
                """psum [128, PA_W or PA3_W] -> tab/att shard rows for block t."""
                SL = SLICES_L[l]
                s = 0
                while SL[s + 1] <= t:
                    s += 1
                t_in_s = t - SL[s]
                if l < 3:
                    tab_sb = wpool.tile([128, TABW], F16, tag="tab_sb")
                    for h in range(H):
                        nc.scalar.copy(tab_sb[:, h * CH1: h * CH1 + C],
                                       psum[:, h * C:(h + 1) * C])
                        nc.vector.memset(tab_sb[:, h * CH1 + C: (h + 1) * CH1], 1.0)
                    nc.scalar.copy(tab_sb[:, ASRC_OFF:ASRC_OFF + 3],
                                   psum[:, D1:D1 + 3])
                    nc.vector.memset(tab_sb[:, ASRC_OFF + 3: TABW], 0.0)
                    adst_lo = D1 + 3
                else:
                    tab_sb = wpool.tile([128, TAB3W], F16, tag="tab3_sb")
                    hoc = H * OC
                    for h in range(H):
                        nc.scalar.copy(tab_sb[:, h * CH3: h * CH3 + OC],
                                       psum[:, h * OC:(h + 1) * OC])
                        nc.vector.memset(tab_sb[:, h * CH3 + OC: (h + 1) * CH3], 1.0)
                    nc.scalar.copy(tab_sb[:, A3_OFF:A3_OFF + 3],
                                   psum[:, hoc:hoc + 3])
                    nc.vector.memset(tab_sb[:, A3_OFF + 3: TAB3W], 0.0)
                    adst_lo = H * OC + 3
                nc.scalar.copy(att_all[l - 1][:, t, :], psum[:, adst_lo:adst_lo + 3])
                nc.sync.dma_start(
                    out=tabsh[l - 1][s][t_in_s * BLK:(t_in_s + 1) * BLK, :],
                    in_=tab_sb[...])

            def phaseA(l, t):
                if l == 1:
                    psum = psA.tile([128, PA_W], F32, tag="psA")
                    nc.tensor.matmul(psum[:, 0:512], xT[:, t, :], wx1[:, 0, 0:512],
                                     start=True, stop=True)
                    nc.tensor.matmul(psum[:, 512:PA_W], xT[:, t, :], wx1[:, 0, 512:PA_W],
                                     start=True, stop=True)
                elif l == 2:
                    psum = psA.tile([128, PA_W], F32, tag="psA")
                    for ct in range(n_cin):
                        nc.tensor.matmul(psum[:, 0:512], hT[:, t * n_cin + ct, :],
                                         wx2[:, ct, 0:512],
                                         start=(ct == 0), stop=(ct == n_cin - 1))
                        nc.tensor.matmul(psum[:, 512:PA_W], hT[:, t * n_cin + ct, :],
                                         wx2[:, ct, 512:PA_W],
                                         start=(ct == 0), stop=(ct == n_cin - 1))
                else:
                    psum_full = psA.tile([128, PA_W], F32, tag="psA")
                    psum = psum_full[:, 0:PA3_W]
                    for ct in range(n_cin):
                        nc.tensor.matmul(psum[:, :], hT[:, t * n_cin + ct, :],
                                         wx3[:, ct, :],
                                         start=(ct == 0), stop=(ct == n_cin - 1))
                phaseA_tail(l, t, psum)

            def allgather_slice(l, s):
                SL = SLICES_L[l]
                nc.gpsimd.collective_compute(
                    "AllGather", ALU.bypass, ins=[tabsh[l - 1][s].ap().opt()],
                    outs=[tabfull[l - 1][
                        SL[s] * NCORES * BLK:SL[s + 1] * NCORES * BLK, :]],
                    replica_groups=RG)

            # ---------------- scatter (+ next layer's phase A interleaved)
            def scatter(l, next_phase, next_l):
                """next_phase(j) interleaved per block; after the last block of
                AG slice s of the NEXT layer, issue that slice's AllGather."""
                tab = tabfull[l - 1]
                tw = TABW if l < 3 else TAB3W
                ch = CH1 if l < 3 else CH3
                cdim = C if l < 3 else OC
                a_off = ASRC_OFF if l < 3 else A3_OFF
                att_l = att_all[l - 1]
                ps = None
                for gi in range(ngrp):
                    c0 = gi * ggrp
                    ng = min(ggrp, ncht - c0)
                    rows = rpool.tile([128, ggrp, tw], F16,
                                      tag="rows_g" if l < 3 else "rows3_g")
                    for s0 in range(0, ng, 8):
                        sn = min(8, ng - s0)
                        nc.gpsimd.dma_gather(
                            out_ap=rows[:, s0:s0 + sn, :], in_ap=tab[:, :],
                            idxs_ap=srcidx[:, l - 1,
                                           (c0 + s0) * 8:(c0 + s0 + sn) * 8],
                            num_idxs=sn * BLK, num_idxs_reg=sn * BLK,
                            elem_size=tw, queue_num=next_q())
                    # one-hot masks for the group (static DMA)
                    oh_e = opool.tile([128, ggrp, BLK], F8, tag="oh_e")
                    nc.sync.dma_start(out=oh_e[:, 0:ng, :],
                                      in_=oh_e_in[:, c0:c0 + ng, :])
                    # ex for the whole group: exp(leaky(asrc + adst))
                    exg = wpool.tile([128, ggrp, 3], F32, tag="exg")
                    tm = wpool.tile([128, ggrp, 3], F32, tag="exg_t")
                    eacc = wpool.tile([128, ggrp, 3], F32, tag="eacc")
                    nc.vector.tensor_tensor(
                        out=eacc[:, 0:ng, :], in0=rows[:, 0:ng, a_off:a_off + 3],
                        in1=pse_all[:, c0:c0 + ng, :], op=ALU.add)
                    nc.vector.tensor_scalar_min(tm[:, 0:ng, :], eacc[:, 0:ng, :], 0.0)
                    nc.vector.tensor_scalar_max(eacc[:, 0:ng, :], eacc[:, 0:ng, :], 0.0)
                    nc.vector.scalar_tensor_tensor(
                        out=tm[:, 0:ng, :], in0=tm[:, 0:ng, :],
                        scalar=NEG_SLOPE, in1=eacc[:, 0:ng, :],
                        op0=ALU.mult, op1=ALU.add)
                    nc.scalar.activation(exg[:, 0:ng, :], tm[:, 0:ng, :], AF.Exp)
                    cc = 0
                    while cc < ng:
                        take = 4 if cc + 3 < ng else (2 if cc + 1 < ng else 1)
                        m3 = m3p.tile([128, 4 * H, 128], F16, tag="m3")
                        # m3_h = onehot * ex_h, `take` chunks x 3 heads per op
                        if take > 1:
                            nc.vector.tensor_tensor(
                                out=m3[:, 0:take * H, :],
                                in0=oh_e[:, cc:cc + take, :].unsqueeze(2)
                                    .broadcast_to([128, take, H, 128]),
                                in1=exg[:, cc:cc + take, :].unsqueeze(3)
                                    .broadcast_to([128, take, H, 128]),
                                op=ALU.mult)
                        else:
                            nc.vector.tensor_tensor(
                                out=m3[:, 0:H, :],
                                in0=oh_e[:, cc, :].unsqueeze(1)
                                    .broadcast_to([128, H, 128]),
                                in1=exg[:, cc, :].unsqueeze(2)
                                    .broadcast_to([128, H, 128]),
                                op=ALU.mult)
                        for q in range(take):
                            c = c0 + cc + q
                            j, cj = divmod(c, nchunk)
                            if cj == 0:
                                ps = [psH.tile([128, ch], F32, tag=f"psH{h}",
                                               name=f"psH{h}") for h in range(H)]
                            for h in range(H):
                                nc.tensor.matmul(
                                    ps[h][:, :], m3[:, q * H + h, :],
                                    rows[:, cc + q, h * ch:(h + 1) * ch],
                                    start=(cj == 0), stop=(cj == nchunk - 1))
                            if cj == nchunk - 1:
                                epilogue(l, j, ps)
                                if next_phase is not None:
                                    next_phase(j)
                                    SLn = SLICES_L[next_l]
                                    for s in range(len(SLn) - 1):
                                        if SLn[s + 1] == j + 1:
                                            allgather_slice(next_l, s)
                        cc += take

            def epilogue(l, j, ps):
                cdim = C if l < 3 else OC
                recip = wpool.tile([128, H], F32, tag="recip")
                den = wpool.tile([128, H], F32, tag="den")
                for h in range(H):
                    nc.vector.tensor_scalar_add(den[:, h:h + 1],
                                                ps[h][:, cdim:cdim + 1], 1e-16)
                nc.vector.reciprocal(recip[...], den[...])
                if l < 3:
                    h_at = wpool.tile([128, D1], F16, tag="h_at")
                    for h in range(H):
                        nc.scalar.activation(h_at[:, h * cdim:(h + 1) * cdim],
                                             ps[h][:, 0:cdim], AF.Copy,
                                             scale=recip[:, h:h + 1])
                    nc.vector.tensor_tensor(out=h_at[...], in0=h_at[...],
                                            in1=b_sb[l - 1][...], op=ALU.add)
                    tmin = wpool.tile([128, D1], F16, tag="tmin")
                    nc.vector.tensor_scalar_min(tmin[...], h_at[...], 0.0)
                    texp = wpool.tile([128, D1], F16, tag="texp")
                    nc.scalar.activation(texp[...], tmin[...], AF.Exp)
                    nc.vector.tensor_scalar_max(h_at[...], h_at[...], 0.0)
                    h_in = wpool.tile([128, D1], F16, tag="h_in")
                    nc.vector.scalar_tensor_tensor(
                        out=h_in[...], in0=texp[...], scalar=-1.0,
                        in1=h_at[...], op0=ALU.add, op1=ALU.add)
                    for ct in range(n_cin):
                        pt = psT.tile([128, 128], F16, tag="psT")
                        nc.tensor.transpose(pt[...],
                                            h_in[:, ct * 128:(ct + 1) * 128],
                                            ident16[...])
                        nc.scalar.copy(hT[:, j * n_cin + ct, :], pt[...])
                else:
                    r3 = wpool.tile([128, H], F32, tag="r3")
                    nc.vector.tensor_scalar_mul(r3[...], recip[...], 1.0 / H)
                    acc = wpool.tile([128, OC], F32, tag="acc")
                    nc.scalar.activation(acc[...], ps[0][:, 0:OC], AF.Copy,
                                         scale=r3[:, 0:1])
                    acc2 = wpool.tile([128, OC], F32, tag="acc2")
                    nc.vector.scalar_tensor_tensor(
                        out=acc2[...], in0=ps[1][:, 0:OC], scalar=r3[:, 1:2],
                        in1=acc[...], op0=ALU.mult, op1=ALU.add)
                    nc.vector.scalar_tensor_tensor(
                        out=acc[...], in0=ps[2][:, 0:OC], scalar=r3[:, 2:3],
                        in1=acc2[...], op0=ALU.mult, op1=ALU.add)
                    nc.vector.tensor_tensor(out=acc[...], in0=acc[...],
                                            in1=b_sb[2][...], op=ALU.add)
                    tmin = wpool.tile([128, OC], F32, tag="tmin3")
                    nc.vector.tensor_scalar_min(tmin[...], acc[...], 0.0)
                    texp = wpool.tile([128, OC], F32, tag="texp3")
                    nc.scalar.activation(texp[...], tmin[...], AF.Exp)
                    nc.vector.tensor_scalar_max(acc[...], acc[...], 0.0)
                    fin = wpool.tile([128, OC], F32, tag="fin")
                    nc.vector.scalar_tensor_tensor(
                        out=fin[...], in0=texp[...], scalar=-1.0,
                        in1=acc[...], op0=ALU.add, op1=ALU.add)
                    nc.sync.dma_start(out=out_ext[j * BLK:(j + 1) * BLK, :],
                                      in_=fin[...])

            # ================= main flow =================
            SL1 = SLICES_L[1]
            for t in range(NBLK):
                phaseA(1, t)
                for s in range(len(SL1) - 1):
                    if SL1[s + 1] == t + 1:
                        allgather_slice(1, s)
            adst_precompute(1)
            scatter(1, lambda j: phaseA(2, j), 2)
            adst_precompute(2)
            scatter(2, lambda j: phaseA(3, j), 3)
            adst_precompute(3)
            scatter(3, None, None)

    nc.compile()
    return nc


# ------------------------------------------------------------------ driver
_CACHE = {}


def _get_program(cfg_key, cfg, nchunk):
    key = (cfg_key, nchunk)
    if key not in _CACHE:
        _CACHE[key] = build_program(cfg, nchunk)
    return _CACHE[key]


def kernel(x, edge_index, W1, a_src1, a_dst1, b1, W2, a_src2, a_dst2, b2,
           W3, a_src3, a_dst3, b3, _trace=False, _trace_kwargs=None):
    cfg = CFG_FULL
    x = np.asarray(x, np.float32)
    edge_index = np.asarray(edge_index)
    in_maps, nchunk = prep_host(
        x, edge_index,
        [np.asarray(W1, np.float32), np.asarray(W2, np.float32), np.asarray(W3, np.float32)],
        [np.asarray(a_src1, np.float32), np.asarray(a_src2, np.float32), np.asarray(a_src3, np.float32)],
        [np.asarray(a_dst1, np.float32), np.asarray(a_dst2, np.float32), np.asarray(a_dst3, np.float32)],
        [np.asarray(b1, np.float32), np.asarray(b2, np.float32), np.asarray(b3, np.float32)],
        cfg)
    for attempt in range(2):
        try:
            nc = _get_program("full", cfg, nchunk)
            res = run_bass_kernel_spmd(nc, in_maps,
                                       core_ids=list(range(cfg["NCORES"])),
                                       trace=_trace, **(_trace_kwargs or {}))
            out = np.concatenate(
                [res.results[k]["out"] for k in range(cfg["NCORES"])], 0)
            kernel.last_results = res
            return out[:cfg["N"]].astype(np.float32)
        except Exception:
            import traceback
            traceback.print_exc(file=sys.stderr)
            print(f"WARNING: bass kernel attempt {attempt} failed", file=sys.stderr)
    print("WARNING: bass kernel failed; falling back to numpy", file=sys.stderr)
    return _numpy_gat(x, edge_index,
                      [W1, W2, W3], [a_src1, a_src2, a_src3],
                      [a_dst1, a_dst2, a_dst3], [b1, b2, b3])


def _numpy_gat(x, ei, Ws, asrcs, adsts, bs):
    N = x.shape[0]
    loop = np.arange(N, dtype=np.int64)
    src = np.concatenate([ei[0].astype(np.int64), loop])
    dst = np.concatenate([ei[1].astype(np.int64), loop])

    def layer(h_in, W, a_s, a_d, b, concat):
        H, Ch = a_s.shape
        h = (h_in @ W).reshape(N, H, Ch)
        al_s = (h * a_s[None]).sum(-1)
        al_d = (h * a_d[None]).sum(-1)
        e = al_s[src] + al_d[dst]
        e = np.where(e > 0, e, NEG_SLOPE * e)
        m = np.full((N, H), -1e30, np.float32)
        np.maximum.at(m, dst, e)
        ex = np.exp(e - m[dst])
        den = np.zeros((N, H), np.float32)
        np.add.at(den, dst, ex)
        alpha = ex / (den[dst] + 1e-16)
        out = np.zeros_like(h)
        np.add.at(out, dst, alpha[:, :, None] * h[src])
        out = out.reshape(N, -1) if concat else out.mean(1)
        return out + b

    def elu(v):
        return np.where(v > 0, v, np.exp(np.minimum(v, 0)) - 1)

    h = elu(layer(np.asarray(x, np.float32), Ws[0], asrcs[0], adsts[0], bs[0], True))
    h = elu(layer(h, Ws[1], asrcs[1], adsts[1], bs[1], True))
    return elu(layer(h, Ws[2], asrcs[2], adsts[2], bs[2], False)).astype(np.float32)


# revision 74
# speedup vs baseline: 1.0373x; 1.0373x over previous
"""3-layer GAT on 8 Trainium2 NeuronCores (v4).

Strategy (dst-sharded, edge-parallel within core):
- Host: add self-loops, sort edges by dst, partition dst nodes into 8
  contiguous shards (3840/core, 30 blocks of 128). Per dst-block, pad the
  edge list to a uniform chunk count (128 edges/chunk).
- Host precomputes the per-chunk one-hot masks onehot[e,d] (f8, exact 0/1)
  and their transposes onehotT[d,e]; both stream from DRAM (static DMA).
  No per-chunk is_equal builds on device.
- Device, per layer: phase A computes h_ext = x @ [W | wsrc | wdst] for the
  core's own node shard. The message-table row (1792B for layers 1/2) holds
  h head-blocks in f16 with interleaved 1.0 columns plus asrc; adst for the
  core's own dst nodes stays in SBUF (att_all tile).
- The message table is AllGathered into a pair-Shared DRAM tensor (~2x the
  bandwidth of Local-output collectives, one copy per HBM pair) in
  per-layer slices (SLICES_L); slices for layer l+1 are issued from inside
  scatter l right after the contributing phase-A blocks complete, so the
  collectives hide under the scatter. tabfull uses a slice-blocked row
  layout; the host remaps src indices per layer accordingly.
- Per-edge adst (pse_all) = onehotT.T @ att_blk, ALL chunks precomputed in
  the AllGather shadow between scatters (3-col matmuls, psum -> f16 SBUF).
- Scatter: per 8-chunk group, dma_gather the src rows (4 SWDGE queues,
  round-robin); ex = exp(leakyrelu(asrc+pse)) per group; per 128-edge chunk
  ONE stride-0-broadcast DVE multiply builds m3[e,(h,d)] = onehot*ex_h for
  all 3 heads, then psum_h += m3_h.T @ rows[h-block|1]. den comes from the
  interleaved ones column. Normalize, +bias, ELU; transpose via PE into
  f16 hT for the next layer's phase A.
"""
import sys, os
sys.path.insert(0, "/opt/trn_rl_repo")
import math
import numpy as np

from concourse import bacc, tile, mybir
from concourse.bass_utils import run_bass_kernel_spmd

F32 = mybir.dt.float32
F16 = mybir.dt.float16
F8 = mybir.dt.float8e4
I16 = mybir.dt.int16
I32 = mybir.dt.int32
AF = mybir.ActivationFunctionType
ALU = mybir.AluOpType

NEG_SLOPE = 0.2
# phase-A block ranges per AllGather slice, per layer
SLICES_L = {
    1: [0, 6, 12, 18, 24, 30],
    2: [0, 2, 5, 9, 13, 17, 21, 25, 28, 30],
    3: [0, 20, 28, 30],
}


def make_cfg(N, F_IN, H, C, OC, NCORES=8):
    cfg = dict(N=N, F_IN=F_IN, H=H, C=C, OC=OC, NCORES=NCORES)
    D1 = H * C
    blk = 128
    npc = math.ceil(N / (NCORES * blk)) * blk  # nodes per core
    cfg.update(
        D1=D1,
        BLK=blk,
        NPC=npc,
        NBLK=npc // blk,
        NPAD=npc * NCORES,
        TABW=896,        # layers 1/2 row: f16 elems (1792B)
        TAB3W=128,       # layer 3 row: f16 elems (256B)
        ASRC_OFF=771,    # f16 offset of asrc in the l1/l2 row
        A3_OFF=99,       # f16 offset of asrc in the l3 row
        PA_W=D1 + 6,     # phase-A psum width (h | asrc | adst)
        PA3_W=H * OC + 6,
    )
    return cfg


CFG_FULL = make_cfg(N=30000, F_IN=128, H=3, C=256, OC=32)


# ---------------------------------------------------------------- host prep
def prep_host(x, edge_index, Ws, asrcs, adsts, bs, cfg):
    """Returns (in_maps, nchunk)."""
    N, H, C, OC = cfg["N"], cfg["H"], cfg["C"], cfg["OC"]
    NCORES, NPC, NBLK, BLK = cfg["NCORES"], cfg["NPC"], cfg["NBLK"], cfg["BLK"]

    src = edge_index[0].astype(np.int64)
    dst = edge_index[1].astype(np.int64)
    loop = np.arange(N, dtype=np.int64)
    src = np.concatenate([src, loop])
    dst = np.concatenate([dst, loop])

    order = np.argsort(dst, kind="stable")
    src_s, dst_s = src[order], dst[order]
    gb = dst_s // BLK                       # global block id
    nblk_g = NCORES * NBLK
    counts = np.bincount(gb, minlength=nblk_g)
    nchunk = max(1, int(math.ceil(counts.max() / BLK)))
    spb = nchunk * BLK                      # slots per block
    slots = NBLK * spb                      # per core
    ncht = NBLK * nchunk

    offsets = np.zeros(nblk_g, np.int64)
    offsets[1:] = np.cumsum(counts)[:-1]
    pos_in_block = np.arange(len(dst_s)) - offsets[gb]
    core_id = gb // NBLK
    loc_blk = gb % NBLK

    # slice-blocked tabfull row remap: node g = c*NPC + b*128 + r ->
    # row = base[s] + c*(len_s*128) + (b-B_s)*BLK + r   (per-layer slices)
    def make_remap(SLICES):
        nsl = len(SLICES) - 1
        slice_of = np.zeros(NBLK, np.int64)
        base = np.zeros(nsl, np.int64)
        slen = np.zeros(nsl, np.int64)
        for s in range(nsl):
            slice_of[SLICES[s]:SLICES[s + 1]] = s
            base[s] = SLICES[s] * NCORES * BLK
            slen[s] = SLICES[s + 1] - SLICES[s]
        sl_arr = np.asarray(SLICES, np.int64)

        def remap(g):
            c = g // NPC
            b = (g % NPC) // BLK
            r = g % BLK
            s = slice_of[b]
            return base[s] + c * (slen[s] * BLK) + (b - sl_arr[s]) * BLK + r
        return remap

    remaps = {l: make_remap(SLICES_L[l]) for l in (1, 2, 3)}

    slot = loc_blk * spb + pos_in_block

    def wrap16(a):
        return np.ascontiguousarray(np.tile(a.reshape(-1, 16).T, (8, 1)))

    in_maps = []
    for k in range(NCORES):
        sel = core_id == k
        sl = slot[sel]
        relf = np.full(slots, 999, np.int64)  # padded: no match
        relf[sl] = dst_s[sel] % BLK
        srcs = []
        for l in (1, 2, 3):
            srcf = np.zeros(slots, np.int64)
            srcf[sl] = remaps[l](src_s[sel])
            srcs.append(wrap16(srcf.astype(np.int16)))
        srcidx3 = np.ascontiguousarray(np.stack(srcs, axis=1))  # [128,3,slots//16]

        # one-hot masks, f8e4m3 exact 0/1 (1.0 = 0x38): onehot[e,d], onehotT[d,e]
        import ml_dtypes
        oh = (relf.reshape(ncht, BLK)[:, :, None] ==
              np.arange(BLK)[None, None, :])  # [chunk, e, d]
        oh_e = np.ascontiguousarray(
            np.where(oh.transpose(1, 0, 2), np.uint8(0x38), np.uint8(0))
        ).view(ml_dtypes.float8_e4m3)
        oh_d = np.ascontiguousarray(
            np.where(oh.transpose(2, 0, 1), np.uint8(0x38), np.uint8(0))
        ).view(ml_dtypes.float8_e4m3)

        xk = np.zeros((NPC, cfg["F_IN"]), np.float32)
        lo, hi = k * NPC, min((k + 1) * NPC, N)
        if hi > lo:
            xk[: hi - lo] = x[lo:hi]
        xT = np.ascontiguousarray(
            xk.reshape(NBLK, BLK, cfg["F_IN"]).transpose(2, 0, 1)
        ).astype(np.float16)

        m = dict(
            xT_in=xT,
            srcidx_in=srcidx3,
            oh_e_in=oh_e,
            oh_d_in=oh_d,
        )
        in_maps.append(m)

    # weights (replicated): wx = [W | wsrc | wdst] in f16
    def wx(W, a_s, a_d):
        Ch = W.shape[1] // H
        Wr = W.reshape(W.shape[0], H, Ch)
        ws = np.einsum("khc,hc->kh", Wr, a_s)
        wd = np.einsum("khc,hc->kh", Wr, a_d)
        return np.concatenate([W, ws, wd], axis=1).astype(np.float16)

    wx1 = wx(Ws[0], asrcs[0], adsts[0])
    wx2 = wx(Ws[1], asrcs[1], adsts[1])
    wx3 = wx(Ws[2], asrcs[2], adsts[2])
    b1 = np.broadcast_to(bs[0], (128, cfg["D1"])).astype(np.float16).copy()
    b2 = np.broadcast_to(bs[1], (128, cfg["D1"])).astype(np.float16).copy()
    b3 = np.broadcast_to(bs[2], (128, OC)).astype(np.float32).copy()
    for m in in_maps:
        m.update(wx1_in=wx1, wx2_in=wx2, wx3_in=wx3, b1_in=b1, b2_in=b2, b3_in=b3)
    return in_maps, nchunk


# ------------------------------------------------------------- device build
def build_program(cfg, nchunk, ggrp=16, nq=4):
    N, F_IN, H, C, OC = cfg["N"], cfg["F_IN"], cfg["H"], cfg["C"], cfg["OC"]
    D1, NCORES, NPC, NBLK, BLK = (
        cfg["D1"], cfg["NCORES"], cfg["NPC"], cfg["NBLK"], cfg["BLK"])
    NPAD = cfg["NPAD"]
    TABW, TAB3W = cfg["TABW"], cfg["TAB3W"]
    ASRC_OFF, A3_OFF = cfg["ASRC_OFF"], cfg["A3_OFF"]
    PA_W, PA3_W = cfg["PA_W"], cfg["PA3_W"]
    CH1 = C + 1                     # f16 head stride in l1/l2 row
    CH3 = OC + 1                    # f16 head stride in l3 row
    ncht = NBLK * nchunk
    slots = ncht * BLK
    n_cin = D1 // 128
    ngrp = math.ceil(ncht / ggrp)   # gather groups per layer

    nc = bacc.Bacc("TRN2", target_bir_lowering=False, debug=False,
                   num_devices=NCORES, num_swdge_queues=nq)
    qrr = [0]

    def next_q():
        q = qrr[0] % nq
        qrr[0] += 1
        return q

    # ---- I/O
    xT_in = nc.dram_tensor("xT_in", [128, NBLK, F_IN], F16, kind="ExternalInput")
    srcidx_in = nc.dram_tensor("srcidx_in", [128, 3, slots // 16], I16, kind="ExternalInput")
    oh_e_in = nc.dram_tensor("oh_e_in", [128, ncht, BLK], F8, kind="ExternalInput")
    oh_d_in = nc.dram_tensor("oh_d_in", [128, ncht, BLK], F8, kind="ExternalInput")
    wx1_in = nc.dram_tensor("wx1_in", [F_IN, PA_W], F16, kind="ExternalInput")
    wx2_in = nc.dram_tensor("wx2_in", [D1, PA_W], F16, kind="ExternalInput")
    wx3_in = nc.dram_tensor("wx3_in", [D1, PA3_W], F16, kind="ExternalInput")
    b1_in = nc.dram_tensor("b1_in", [128, D1], F16, kind="ExternalInput")
    b2_in = nc.dram_tensor("b2_in", [128, D1], F16, kind="ExternalInput")
    b3_in = nc.dram_tensor("b3_in", [128, OC], F32, kind="ExternalInput")
    out_ext = nc.dram_tensor("out", [NPC, OC], F32, kind="ExternalOutput")

    # ---- DRAM scratch: per-slice shard inputs, pair-shared gathered tables
    def mk_tabsh(l, w):
        SL = SLICES_L[l]
        return [nc.dram_tensor(f"tabsh{l}_{s}",
                               [(SL[s + 1] - SL[s]) * BLK, w], F16)
                for s in range(len(SL) - 1)]

    tabsh = [mk_tabsh(1, TABW), mk_tabsh(2, TABW), mk_tabsh(3, TAB3W)]
    tabfull = [nc.dram_tensor("tabfull1", [NPAD, TABW], F16, addr_space="Shared"),
               nc.dram_tensor("tabfull2", [NPAD, TABW], F16, addr_space="Shared"),
               nc.dram_tensor("tabfull3", [NPAD, TAB3W], F16, addr_space="Shared")]
    RG = [list(range(NCORES))]

    with tile.TileContext(nc) as tc:
        with (
            tc.tile_pool(name="const", bufs=1) as cpool,
            tc.tile_pool(name="rows", bufs=2) as rpool,
            tc.tile_pool(name="oh", bufs=2) as opool,
            tc.tile_pool(name="work", bufs=2) as wpool,
            tc.tile_pool(name="psA", bufs=1, space="PSUM") as psA,
            tc.tile_pool(name="psH", bufs=1, space="PSUM") as psH,
            tc.tile_pool(name="psT", bufs=1, space="PSUM") as psT,
            tc.tile_pool(name="psE", bufs=2, space="PSUM") as psE,
            tc.tile_pool(name="m3p", bufs=4) as m3p,
            tc.tile_pool(name="ohd", bufs=1) as ohdp,
        ):
            # ---------------- constants
            def load_const(name, dram, shape, dtype):
                t = cpool.tile(shape, dtype, tag=name)
                nc.sync.dma_start(out=t[...], in_=dram[...])
                return t

            xT = load_const("xT", xT_in, [128, NBLK, F_IN], F16)
            srcidx = load_const("srcidx", srcidx_in, [128, 3, slots // 16], I16)
            b_sb = [load_const("b1", b1_in, [128, D1], F16),
                    load_const("b2", b2_in, [128, D1], F16),
                    load_const("b3", b3_in, [128, OC], F32)]
            wx1 = cpool.tile([128, 1, PA_W], F16, tag="wx1")
            nc.sync.dma_start(out=wx1[:, 0, :], in_=wx1_in[0:128, :])
            wx2 = cpool.tile([128, n_cin, PA_W], F16, tag="wx2")
            wx3 = cpool.tile([128, n_cin, PA3_W], F16, tag="wx3")
            for ct in range(n_cin):
                nc.sync.dma_start(out=wx2[:, ct, :], in_=wx2_in[ct * 128:(ct + 1) * 128, :])
                nc.sync.dma_start(out=wx3[:, ct, :], in_=wx3_in[ct * 128:(ct + 1) * 128, :])

            iota_i = cpool.tile([128, 128], I32, tag="iota_i")
            nc.gpsimd.iota(iota_i[...], pattern=[[1, 128]], base=0, channel_multiplier=0)
            iota16 = cpool.tile([128, 128], F16, tag="iota16")
            nc.vector.tensor_copy(iota16[...], iota_i[...])
            pidx_i = cpool.tile([128, 1], I32, tag="pidx_i")
            nc.gpsimd.iota(pidx_i[...], pattern=[[0, 1]], base=0, channel_multiplier=1)
            pidx_f = cpool.tile([128, 1], F32, tag="pidx_f")
            nc.vector.tensor_copy(pidx_f[...], pidx_i[...])
            ident16 = cpool.tile([128, 128], F16, tag="ident16")
            nc.vector.tensor_scalar(out=ident16[...], in0=iota16[...],
                                    scalar1=pidx_f[:, 0:1], scalar2=None,
                                    op0=ALU.is_equal)

            # persistent f16 transposed activations for next layer's phase A
            hT = cpool.tile([128, n_cin * NBLK, 128], F16, tag="hT")
            # per-layer adst of the core's own dst blocks [dstrow, (block, h)]
            att_all = [cpool.tile([128, NBLK, 3], F16, tag=f"att{l}",
                                  name=f"att{l}") for l in (1, 2, 3)]
            # per-edge adst for the CURRENT layer, precomputed between scatters
            pse_all = cpool.tile([128, ncht, 3], F16, tag="pse_all")

            # pse_all[e, c, h] = sum_d onehotT[d, c, e] * att[d, c//nchunk, h]
            def adst_precompute(l):
                att_l = att_all[l - 1]
                CPR = 128  # chunks per psum round (128*3 = 384 f32 <= bank)
                for r0 in range(0, ncht, CPR):
                    rn = min(CPR, ncht - r0)
                    ohd = ohdp.tile([128, CPR, BLK], F8, tag="ohd_r")
                    nc.scalar.dma_start(out=ohd[:, 0:rn, :],
                                        in_=oh_d_in[:, r0:r0 + rn, :])
                    pp = psE.tile([128, CPR, 3], F32, tag="pse_ps")
                    for i in range(rn):
                        j = (r0 + i) // nchunk
                        nc.tensor.matmul(pp[:, i, :], ohd[:, i, :],
                                         att_l[:, j, :], start=True, stop=True)
                    nc.scalar.copy(pse_all[:, r0:r0 + rn, :], pp[:, 0:rn, :])

            # ---------------- phase A
            def phaseA_tail(l, t, psum):
                """psum [128, PA_W or PA3_W] -> tab/att shard rows for block t."""
                SL = SLICES_L[l]
                s = 0
                while SL[s + 1] <= t:
                    s += 1
                t_in_s = t - SL[s]
                if l < 3:
                    tab_sb = wpool.tile([128, TABW], F16, tag="tab_sb")
                    for h in range(H):
                        nc.scalar.copy(tab_sb[:, h * CH1: h * CH1 + C],
                                       psum[:, h * C:(h + 1) * C])
                        nc.vector.memset(tab_sb[:, h * CH1 + C: (h + 1) * CH1], 1.0)
                    nc.scalar.copy(tab_sb[:, ASRC_OFF:ASRC_OFF + 3],
                                   psum[:, D1:D1 + 3])
                    nc.vector.memset(tab_sb[:, ASRC_OFF + 3: TABW], 0.0)
                    adst_lo = D1 + 3
                else:
                    tab_sb = wpool.tile([128, TAB3W], F16, tag="tab3_sb")
                    hoc = H * OC
                    for h in range(H):
                        nc.scalar.copy(tab_sb[:, h * CH3: h * CH3 + OC],
                                       psum[:, h * OC:(h + 1) * OC])
                        nc.vector.memset(tab_sb[:, h * CH3 + OC: (h + 1) * CH3], 1.0)
                    nc.scalar.copy(tab_sb[:, A3_OFF:A3_OFF + 3],
                                   psum[:, hoc:hoc + 3])
                    nc.vector.memset(tab_sb[:, A3_OFF + 3: TAB3W], 0.0)
                    adst_lo = H * OC + 3
                nc.scalar.copy(att_all[l - 1][:, t, :], psum[:, adst_lo:adst_lo + 3])
                nc.sync.dma_start(
                    out=tabsh[l - 1][s][t_in_s * BLK:(t_in_s + 1) * BLK, :],
                    in_=tab_sb[...])

            def phaseA(l, t):
                if l == 1:
                    psum = psA.tile([128, PA_W], F32, tag="psA")
                    nc.tensor.matmul(psum[:, 0:512], xT[:, t, :], wx1[:, 0, 0:512],
                                     start=True, stop=True)
                    nc.tensor.matmul(psum[:, 512:PA_W], xT[:, t, :], wx1[:, 0, 512:PA_W],
                                     start=True, stop=True)
                elif l == 2:
                    psum = psA.tile([128, PA_W], F32, tag="psA")
                    for ct in range(n_cin):
                        nc.tensor.matmul(psum[:, 0:512], hT[:, t * n_cin + ct, :],
                                         wx2[:, ct, 0:512],
                                         start=(ct == 0), stop=(ct == n_cin - 1))
                        nc.tensor.matmul(psum[:, 512:PA_W], hT[:, t * n_cin + ct, :],
                                         wx2[:, ct, 512:PA_W],
                                         start=(ct == 0), stop=(ct == n_cin - 1))
                else:
                    psum_full = psA.tile([128, PA_W], F32, tag="psA")
                    psum = psum_full[:, 0:PA3_W]
                    for ct in range(n_cin):
                        nc.tensor.matmul(psum[:, :], hT[:, t * n_cin + ct, :],
                                         wx3[:, ct, :],
                                         start=(ct == 0), stop=(ct == n_cin - 1))
                phaseA_tail(l, t, psum)

            def allgather_slice(l, s):
                SL = SLICES_L[l]
                nc.gpsimd.collective_compute(
                    "AllGather", ALU.bypass, ins=[tabsh[l - 1][s].ap().opt()],
                    outs=[tabfull[l - 1][
                        SL[s] * NCORES * BLK:SL[s + 1] * NCORES * BLK, :]],
                    replica_groups=RG)

            # ---------------- scatter (+ next layer's phase A interleaved)
            def scatter(l, next_phase, next_l):
                """next_phase(j) interleaved per block; after the last block of
                AG slice s of the NEXT layer, issue that slice's AllGather."""
                tab = tabfull[l - 1]
                tw = TABW if l < 3 else TAB3W
                ch = CH1 if l < 3 else CH3
                cdim = C if l < 3 else OC
                a_off = ASRC_OFF if l < 3 else A3_OFF
                att_l = att_all[l - 1]
                ps = None
                for gi in range(ngrp):
                    c0 = gi * ggrp
                    ng = min(ggrp, ncht - c0)
                    rows = rpool.tile([128, ggrp, tw], F16,
                                      tag="rows_g" if l < 3 else "rows3_g")
                    for s0 in range(0, ng, 8):
                        sn = min(8, ng - s0)
                        nc.gpsimd.dma_gather(
                            out_ap=rows[:, s0:s0 + sn, :], in_ap=tab[:, :],
                            idxs_ap=srcidx[:, l - 1,
                                           (c0 + s0) * 8:(c0 + s0 + sn) * 8],
                            num_idxs=sn * BLK, num_idxs_reg=sn * BLK,
                            elem_size=tw, queue_num=next_q())
                    # one-hot masks for the group (static DMA)
                    oh_e = opool.tile([128, ggrp, BLK], F8, tag="oh_e")
                    nc.sync.dma_start(out=oh_e[:, 0:ng, :],
                                      in_=oh_e_in[:, c0:c0 + ng, :])
                    # ex for the whole group: exp(leaky(asrc + adst))
                    exg = wpool.tile([128, ggrp, 3], F32, tag="exg")
                    tm = wpool.tile([128, ggrp, 3], F32, tag="exg_t")
                    eacc = wpool.tile([128, ggrp, 3], F32, tag="eacc")
                    nc.vector.tensor_tensor(
                        out=eacc[:, 0:ng, :], in0=rows[:, 0:ng, a_off:a_off + 3],
                        in1=pse_all[:, c0:c0 + ng, :], op=ALU.add)
                    nc.vector.tensor_scalar_min(tm[:, 0:ng, :], eacc[:, 0:ng, :], 0.0)
                    nc.vector.tensor_scalar_max(eacc[:, 0:ng, :], eacc[:, 0:ng, :], 0.0)
                    nc.vector.scalar_tensor_tensor(
                        out=tm[:, 0:ng, :], in0=tm[:, 0:ng, :],
                        scalar=NEG_SLOPE, in1=eacc[:, 0:ng, :],
                        op0=ALU.mult, op1=ALU.add)
                    nc.scalar.activation(exg[:, 0:ng, :], tm[:, 0:ng, :], AF.Exp)
                    cc = 0
                    while cc < ng:
                        take = 2 if cc + 1 < ng else 1
                        m3 = m3p.tile([128, 2 * H, 128], F16, tag="m3")
                        # m3_h = onehot * ex_h, `take` chunks x 3 heads per op
                        if take > 1:
                            nc.vector.tensor_tensor(
                                out=m3[:, 0:take * H, :],
                                in0=oh_e[:, cc:cc + take, :].unsqueeze(2)
                                    .broadcast_to([128, take, H, 128]),
                                in1=exg[:, cc:cc + take, :].unsqueeze(3)
                                    .broadcast_to([128, take, H, 128]),
                                op=ALU.mult)
                        else:
                            nc.vector.tensor_tensor(
                                out=m3[:, 0:H, :],
                                in0=oh_e[:, cc, :].unsqueeze(1)
                                    .broadcast_to([128, H, 128]),
                                in1=exg[:, cc, :].unsqueeze(2)
                                    .broadcast_to([128, H, 128]),
                                op=ALU.mult)
                        for q in range(take):
                            c = c0 + cc + q
                            j, cj = divmod(c, nchunk)
                            if cj == 0:
                                ps = [psH.tile([128, ch], F32, tag=f"psH{h}",
                                               name=f"psH{h}") for h in range(H)]
                            for h in range(H):
                                nc.tensor.matmul(
                                    ps[h][:, :], m3[:, q * H + h, :],
                                    rows[:, cc + q, h * ch:(h + 1) * ch],
                                    start=(cj == 0), stop=(cj == nchunk - 1))
                            if cj == nchunk - 1:
                                epilogue(l, j, ps)
                                if next_phase is not None:
                                    next_phase(j)
                                    SLn = SLICES_L[next_l]
                                    for s in range(len(SLn) - 1):
                                        if SLn[s + 1] == j + 1:
                                            allgather_slice(next_l, s)
                        cc += take

            def epilogue(l, j, ps):
                cdim = C if l < 3 else OC
                recip = wpool.tile([128, H], F32, tag="recip")
                den = wpool.tile([128, H], F32, tag="den")
                for h in range(H):
                    nc.vector.tensor_scalar_add(den[:, h:h + 1],
                                                ps[h][:, cdim:cdim + 1], 1e-16)
                nc.vector.reciprocal(recip[...], den[...])
                if l < 3:
                    h_at = wpool.tile([128, D1], F16, tag="h_at")
                    for h in range(H):
                        nc.scalar.activation(h_at[:, h * cdim:(h + 1) * cdim],
                                             ps[h][:, 0:cdim], AF.Copy,
                                             scale=recip[:, h:h + 1])
                    nc.vector.tensor_tensor(out=h_at[...], in0=h_at[...],
                                            in1=b_sb[l - 1][...], op=ALU.add)
                    tmin = wpool.tile([128, D1], F16, tag="tmin")
                    nc.vector.tensor_scalar_min(tmin[...], h_at[...], 0.0)
                    texp = wpool.tile([128, D1], F16, tag="texp")
                    nc.scalar.activation(texp[...], tmin[...], AF.Exp)
                    nc.vector.tensor_scalar_max(h_at[...], h_at[...], 0.0)
                    h_in = wpool.tile([128, D1], F16, tag="h_in")
                    nc.vector.scalar_tensor_tensor(
                        out=h_in[...], in0=texp[...], scalar=-1.0,
                        in1=h_at[...], op0=ALU.add, op1=ALU.add)
                    for ct in range(n_cin):
                        pt = psT.tile([128, 128], F16, tag="psT")
                        nc.tensor.transpose(pt[...],
                                            h_in[:, ct * 128:(ct + 1) * 128],
                                            ident16[...])
                        nc.scalar.copy(hT[:, j * n_cin + ct, :], pt[...])
                else:
                    r3 = wpool.tile([128, H], F32, tag="r3")
                    nc.vector.tensor_scalar_mul(r3[...], recip[...], 1.0 / H)
                    acc = wpool.tile([128, OC], F32, tag="acc")
                    nc.scalar.activation(acc[...], ps[0][:, 0:OC], AF.Copy,
                                         scale=r3[:, 0:1])
                    acc2 = wpool.tile([128, OC], F32, tag="acc2")
                    nc.vector.scalar_tensor_tensor(
                        out=acc2[...], in0=ps[1][:, 0:OC], scalar=r3[:, 1:2],
                        in1=acc[...], op0=ALU.mult, op1=ALU.add)
                    nc.vector.scalar_tensor_tensor(
                        out=acc[...], in0=ps[2][:, 0:OC], scalar=r3[:, 2:3],
                        in1=acc2[...], op0=ALU.mult, op1=ALU.add)
                    nc.vector.tensor_tensor(out=acc[...], in0=acc[...],
                                            in1=b_sb[2][...], op=ALU.add)
                    tmin = wpool.tile([128, OC], F32, tag="tmin3")
                    nc.vector.tensor_scalar_min(tmin[...], acc[...], 0.0)
                    texp = wpool.tile([128, OC], F32, tag="texp3")
                    nc.scalar.activation(texp[...], tmin[...], AF.Exp)
                    nc.vector.tensor_scalar_max(acc[...], acc[...], 0.0)
                    fin = wpool.tile([128, OC], F32, tag="fin")
                    nc.vector.scalar_tensor_tensor(
                        out=fin[...], in0=texp[...], scalar=-1.0,
                        in1=acc[...], op0=ALU.add, op1=ALU.add)
                    nc.sync.dma_start(out=out_ext[j * BLK:(j + 1) * BLK, :],
                                      in_=fin[...])

            # ================= main flow =================
            SL1 = SLICES_L[1]
            for t in range(NBLK):
                phaseA(1, t)
                for s in range(len(SL1) - 1):
                    if SL1[s + 1] == t + 1:
                        allgather_slice(1, s)
            adst_precompute(1)
            scatter(1, lambda j: phaseA(2, j), 2)
            adst_precompute(2)
            scatter(2, lambda j: phaseA(3, j), 3)
            adst_precompute(3)
            scatter(3, None, None)

    nc.compile()
    return nc


# ------------------------------------------------------------------ driver
_CACHE = {}


def _get_program(cfg_key, cfg, nchunk):
    key = (cfg_key, nchunk)
    if key not in _CACHE:
        _CACHE[key] = build_program(cfg, nchunk)
    return _CACHE[key]


def kernel(x, edge_index, W1, a_src1, a_dst1, b1, W2, a_src2, a_dst2, b2,
           W3, a_src3, a_dst3, b3, _trace=False, _trace_kwargs=None):
    cfg = CFG_FULL
    x = np.asarray(x, np.float32)
    edge_index = np.asarray(edge_index)
    in_maps, nchunk = prep_host(
        x, edge_index,
        [np.asarray(W1, np.float32), np.asarray(W2, np.float32), np.asarray(W3, np.float32)],
        [np.asarray(a_src1, np.float32), np.asarray(a_src2, np.float32), np.asarray(a_src3, np.float32)],
        [np.asarray(a_dst1, np.float32), np.asarray(a_dst2, np.float32), np.asarray(a_dst3, np.float32)],
        [np.asarray(b1, np.float32), np.asarray(b2, np.float32), np.asarray(b3, np.float32)],
        cfg)
    for attempt in range(2):
        try:
            nc = _get_program("full", cfg, nchunk)
            res = run_bass_kernel_spmd(nc, in_maps,
                                       core_ids=list(range(cfg["NCORES"])),
                                       trace=_trace, **(_trace_kwargs or {}))
            out = np.concatenate(
                [res.results[k]["out"] for k in range(cfg["NCORES"])], 0)
            kernel.last_results = res
            return out[:cfg["N"]].astype(np.float32)
        except Exception:
            import traceback
            traceback.print_exc(file=sys.stderr)
            print(f"WARNING: bass kernel attempt {attempt} failed", file=sys.stderr)
    print("WARNING: bass kernel failed; falling back to numpy", file=sys.stderr)
    return _numpy_gat(x, edge_index,
                      [W1, W2, W3], [a_src1, a_src2, a_src3],
                      [a_dst1, a_dst2, a_dst3], [b1, b2, b3])


def _numpy_gat(x, ei, Ws, asrcs, adsts, bs):
    N = x.shape[0]
    loop = np.arange(N, dtype=np.int64)
    src = np.concatenate([ei[0].astype(np.int64), loop])
    dst = np.concatenate([ei[1].astype(np.int64), loop])

    def layer(h_in, W, a_s, a_d, b, concat):
        H, Ch = a_s.shape
        h = (h_in @ W).reshape(N, H, Ch)
        al_s = (h * a_s[None]).sum(-1)
        al_d = (h * a_d[None]).sum(-1)
        e = al_s[src] + al_d[dst]
        e = np.where(e > 0, e, NEG_SLOPE * e)
        m = np.full((N, H), -1e30, np.float32)
        np.maximum.at(m, dst, e)
        ex = np.exp(e - m[dst])
        den = np.zeros((N, H), np.float32)
        np.add.at(den, dst, ex)
        alpha = ex / (den[dst] + 1e-16)
        out = np.zeros_like(h)
        np.add.at(out, dst, alpha[:, :, None] * h[src])
        out = out.reshape(N, -1) if concat else out.mean(1)
        return out + b

    def elu(v):
        return np.where(v > 0, v, np.exp(np.minimum(v, 0)) - 1)

    h = elu(layer(np.asarray(x, np.float32), Ws[0], asrcs[0], adsts[0], bs[0], True))
    h = elu(layer(h, Ws[1], asrcs[1], adsts[1], bs[1], True))
    return elu(layer(h, Ws[2], asrcs[2], adsts[2], bs[2], False)).astype(np.float32)
